# revision 4
# baseline (speedup 1.0000x reference)
"""CQAttention (context-query attention) Bass kernel for 8 NeuronCores.

Full inputs:  C [64,128,1000] f32, Q [64,128,100] f32, W [64000,1,384] f32
Full output:  [64, 512, 1000] f32

Sharding: pure data-parallel on the batch dim - 8 batches per core.

Per-batch math (D=128, Lc=1000, Lq=100):
  Ct = C.T [Lc,D], Qt = Q.T [Lq,D], w1/w2/w3 = W row blocks [Lc,D]
  U  = w1 + w3*Ct ; v = rowsum(w2*Ct)
  S  = U @ Q + v  (the v term drops out of the row softmax S1)
  S1 = softmax_cols(S) ; S2 = softmax_rows(S)
  A  = S1 @ Qt ; Bm = S1 @ (S2^T @ Ct)
  out = concat([Ct, A, Ct*A, Ct*Bm], 1).T  -> [4D, Lc]

Layout notes:
 - Lc is tiled 8 x 125 with the INTERLEAVED mapping i = p*8 + t (p =
   partition, t = tile) so the W DMA reads 12KB contiguous per partition.
   All intermediate tensors with an Lc axis are kept in the permuted
   (t-major) order; the final output ops unpermute via strided APs.
 - Scores are built transposed (S0T [Lq, Lc]); all big matmuls run with
   bf16 operands (full PE rate at any free size; transposes cost 1
   cycle/row instead of 2 for f32).
 - U^T is built with ONE bf16 PE transpose per tile: uraw = w1 + w3*Ct
   is folded on gpsimd/vector first.
 - s1 normalization: column sums of exp(S0T) via a ones-vector matmul,
   DVE reciprocal (no Ln/Exp round trip - keeps the scalar engine on a
   single activation table), then a K=1 f32r matmul broadcast.
 - S2 path: PE-transpose exp(S0T) tiles (bf16), scale by exp(v), and
   contract with bf16 Ct tiles (ones column appended for the s2 sums).
 - DMA: all loads+stores on the sync hw-DGE queue; W is loaded as a 2D
   [125 x 12KB] pattern. Loads are prefetched two batches ahead so store
   triggers never head-of-line-block the next batch's loads.
"""

import numpy as np

B, D, LC, LQ = 64, 128, 1000, 100
NCORES = 8
NB = B // NCORES   # batches per core
NT = 8             # LC tiles
TL = LC // NT      # 125

_cache = {}


def _build():
    import concourse.bass as bass
    import concourse.tile as tile
    from concourse import bacc, mybir, masks
    from contextlib import ExitStack

    f32 = mybir.dt.float32
    f32r = mybir.dt.float32r
    bf16 = mybir.dt.bfloat16
    AF = mybir.ActivationFunctionType
    ALU = mybir.AluOpType
    AX = mybir.AxisListType

    nc = bacc.Bacc("TRN2", target_bir_lowering=False, debug=False,
                   num_devices=NCORES)
    C_d = nc.dram_tensor("C", [NB, D, LC], f32, kind="ExternalInput").ap()
    Q_d = nc.dram_tensor("Q", [NB, D, LQ], f32, kind="ExternalInput").ap()
    W_d = nc.dram_tensor("W", [NB, LC, 3 * D], f32, kind="ExternalInput").ap()
    O_d = nc.dram_tensor("OUT", [NB, 4 * D, LC], f32, kind="ExternalOutput").ap()

    with tile.TileContext(nc) as tc, ExitStack() as ctx:
        const_pool = ctx.enter_context(tc.tile_pool(name="const", bufs=1))
        ident = const_pool.tile([128, 128], f32)
        masks.make_identity(nc, ident[:])
        identb = const_pool.tile([128, 128], bf16)
        nc.scalar.activation(identb[:], ident[:], AF.Copy)
        ones_f = const_pool.tile([128, 1], f32)
        nc.vector.memset(ones_f[:], 1.0)
        ones_cb = const_pool.tile([128, 1], bf16)
        nc.scalar.activation(ones_cb[:], ones_f[:], AF.Copy)
        ones_rf = const_pool.tile([1, 128], f32)
        nc.vector.memset(ones_rf[:], 1.0)
        ones_row = const_pool.tile([1, 128], f32r)
        nc.scalar.activation(ones_row[:], ones_rf[:], AF.Copy)
        zero_f = const_pool.tile([128, 1], f32)
        nc.vector.memset(zero_f[:], 0.0)

        sb = ctx.enter_context(tc.tile_pool(name="sb", bufs=3))
        small = ctx.enter_context(tc.tile_pool(name="small", bufs=3))
        outp = ctx.enter_context(tc.tile_pool(name="outp", bufs=2))
        tp_ps = ctx.enter_context(tc.tile_pool(name="tp_ps", bufs=3, space="PSUM"))
        mm_ps = ctx.enter_context(tc.tile_pool(name="mm_ps", bufs=5, space="PSUM"))

        def load(b):
            # w_sb[p, t*3D+c] = W[b, p*8+t, c]  (12KB contiguous per partition)
            w_sb = sb.tile([TL, NT * 3 * D], f32, tag="w", name=f"w{b}")
            nc.sync.dma_start(
                w_sb[:], W_d[b].rearrange("(p t) c -> p (t c)", t=NT))
            c_sb = sb.tile([D, LC], f32, tag="c", name=f"c{b}")
            nc.sync.dma_start(c_sb[:], C_d[b])
            q_sb = sb.tile([D, LQ], f32, tag="q", name=f"q{b}")
            nc.sync.dma_start(q_sb[:], Q_d[b])
            # passthrough output rows 0:D = Ct (no compute dependency)
            nc.sync.dma_start(O_d[b, 0:D], c_sb[:])
            return w_sb, c_sb, q_sb

        tiles = {0: load(0)}
        if NB > 1:
            tiles[1] = load(1)

        for b in range(NB):
            w_sb, c_sb, q_sb = tiles.pop(b)
            if b + 2 < NB:
                tiles[b + 2] = load(b + 2)

            # ---- Qt (early: only needs the Q load) ----
            qb = small.tile([D, LQ], bf16, tag="qb", name=f"qb{b}")
            nc.scalar.activation(qb[:], q_sb[:], AF.Copy)
            qtp = tp_ps.tile([LQ, D], bf16, tag="tp", name=f"qtp{b}")
            nc.tensor.transpose(qtp[:], qb[:], identb[:])
            qt_sb = small.tile([LQ, D], bf16, tag="qt", name=f"qt{b}")
            nc.scalar.activation(qt_sb[:], qtp[:], AF.Copy)

            # views with the interleaved Lc mapping  i = p*8 + t
            c_tiles = c_sb[:].rearrange("d (p t) -> d t p", t=NT)  # [D, t, p]
            wv = w_sb[:].rearrange("p (t c) -> p t c", c=3 * D)
            w1 = wv[:, :, 0:D]
            w2 = wv[:, :, D:2 * D]
            w3 = wv[:, :, 2 * D:3 * D]

            # ---- Ct tiles: f32 PE transpose groups of 4 ----
            ct_sb = sb.tile([TL, NT * (D + 1)], bf16, tag="ct", name=f"ct{b}")
            ctv = ct_sb[:].rearrange("p (t c) -> p t c", c=D + 1)
            nc.vector.memset(ctv[:, :, D:D + 1], 1.0)
            w3ct = sb.tile([TL, NT * D], bf16, tag="w3ct", name=f"w3ct{b}")
            w3ctv = w3ct[:].rearrange("p (t c) -> p t c", c=D)
            uraw = sb.tile([TL, NT * D], bf16, tag="uraw", name=f"uraw{b}")
            urawv = uraw[:].rearrange("p (t c) -> p t c", c=D)
            for g in range(2):
                ctp = tp_ps.tile([TL, 4 * D], f32, tag="tp", name=f"ctp{b}_{g}")
                for k in range(4):
                    t = 4 * g + k
                    nc.tensor.transpose(
                        ctp[:, k * D:(k + 1) * D], c_tiles[:, t, :], ident[:])
                ctpv = ctp[:].rearrange("p (k c) -> p k c", c=D)
                gs = slice(4 * g, 4 * g + 4)
                nc.scalar.activation(ctv[:, gs, 0:D], ctpv, AF.Copy)
                nc.vector.tensor_tensor(out=w3ctv[:, gs, :], in0=w3[:, gs, :],
                                        in1=ctpv, op=ALU.mult)
                # uraw = w1 + w3*Ct (gpsimd, SBUF-only operands)
                nc.gpsimd.tensor_tensor(out=urawv[:, gs, :], in0=w1[:, gs, :],
                                        in1=w3ctv[:, gs, :], op=ALU.add)

            # ---- v = rowsum(w2 * Ct); exp(v) ----
            vtmp = sb.tile([TL, NT * D], f32, tag="vtmp", name=f"vtmp{b}")
            vtmpv = vtmp[:].rearrange("p (t c) -> p t c", c=D)
            nc.gpsimd.tensor_tensor(out=vtmpv, in0=w2,
                                    in1=ctv[:, :, 0:D], op=ALU.mult)
            v_all = small.tile([TL, NT], f32, tag="v", name=f"v{b}")
            nc.vector.tensor_reduce(v_all[:], vtmpv, axis=AX.X, op=ALU.add)
            expv = small.tile([TL, NT], f32, tag="expv", name=f"expv{b}")
            nc.scalar.activation(expv[:], v_all[:], AF.Exp)

            # ---- U^T via single bf16 PE transpose per tile ----
            ut_sb = sb.tile([D, 8 * D], bf16, tag="ut", name=f"ut{b}")
            utv = ut_sb[:].rearrange("d (t c) -> d t c", c=D)
            nc.scalar.activation(
                utv[:, :, TL:D],
                zero_f[:, 0:1].to_broadcast((D, NT, D - TL)), AF.Copy)
            e1t_sb = sb.tile([LQ, 8 * D], bf16, tag="e1t", name=f"e1t{b}")
            for g in range(2):
                utp = tp_ps.tile([D, 4 * D], bf16, tag="tp", name=f"utp{b}_{g}")
                for k in range(4):
                    t = 4 * g + k
                    nc.tensor.transpose(
                        utp[:, k * D:k * D + TL],
                        uraw[:, t * D:(t + 1) * D],
                        identb[0:TL, 0:TL])
                nc.scalar.activation(
                    utv[:, 4 * g:4 * g + 4, 0:TL],
                    utp[:].rearrange("d (k c) -> d k c", c=D)[:, :, 0:TL],
                    AF.Copy)
                s0g = mm_ps.tile([LQ, 512], f32, tag="mmh", name=f"s0t{b}_{g}")
                nc.tensor.matmul(s0g[:], qb[:],
                                 ut_sb[:, g * 512:(g + 1) * 512],
                                 start=True, stop=True)
                nc.scalar.activation(e1t_sb[:, g * 512:(g + 1) * 512],
                                     s0g[:], AF.Exp)

            # ---- s1 normalization: 1/colsum(E1T) via DVE reciprocal, then
            #      a K=1 f32r matmul broadcast over the Lq partitions ----
            s1t = sb.tile([LQ, 8 * D], bf16, tag="s1t", name=f"s1t{b}")
            s1ri = small.tile([1, 8 * D], f32r, tag="s1ri", name=f"s1ri{b}")
            for g in range(2):
                ssum = tp_ps.tile([1, 512], f32, tag="tp", name=f"ssum{b}_{g}")
                nc.tensor.matmul(ssum[:], ones_cb[0:LQ, :],
                                 e1t_sb[:, g * 512:(g + 1) * 512],
                                 start=True, stop=True)
                gsl = slice(g * 512, (g + 1) * 512)
                with nc.allow_low_precision(reason="f32r == f32 storage"):
                    nc.vector.reciprocal(s1ri[:, gsl], ssum[:])
                bch = mm_ps.tile([LQ, 512], f32, tag="mmh", name=f"bch{b}_{g}")
                nc.tensor.matmul(bch[:], ones_row[:, 0:LQ], s1ri[:, gsl],
                                 start=True, stop=True)
                nc.vector.tensor_tensor(out=s1t[:, gsl],
                                        in0=e1t_sb[:, gsl], in1=bch[:],
                                        op=ALU.mult)

            # ---- E2 tiles = transpose(E1T) * exp(v) ----
            e2_all = sb.tile([TL, NT * LQ], bf16, tag="e2", name=f"e2{b}")
            e2v = e2_all[:].rearrange("p (t c) -> p t c", c=LQ)
            for g in range(2):
                e1p = tp_ps.tile([TL, 4 * LQ], bf16, tag="tp",
                                 name=f"e1p{b}_{g}")
                for k in range(4):
                    t = 4 * g + k
                    nc.tensor.transpose(
                        e1p[:, k * LQ:(k + 1) * LQ],
                        e1t_sb[:, t * D:t * D + TL],
                        identb[0:LQ, 0:LQ])
                e1pv = e1p[:].rearrange("p (k c) -> p k c", c=LQ)
                scl = expv[:, 4 * g:4 * g + 4].unsqueeze(-1).to_broadcast(
                    (TL, 4, LQ))
                nc.vector.tensor_tensor(
                    out=e2v[:, 4 * g:4 * g + 4, :], in0=e1pv,
                    in1=scl, op=ALU.mult)

            # ---- Tu = E2^T @ [Ct | 1]  (accumulate over tiles) ----
            tu = tp_ps.tile([LQ, D + 1], f32, tag="tp", name=f"tu{b}")
            for t in range(NT):
                nc.tensor.matmul(tu[:], e2v[:, t, :], ctv[:, t, :],
                                 start=(t == 0), stop=(t == NT - 1))
            s2r = small.tile([LQ, 1], f32, tag="s2r", name=f"s2r{b}")
            nc.vector.reciprocal(s2r[:], tu[:, D:D + 1])
            that_sb = small.tile([LQ, D], bf16, tag="that", name=f"that{b}")
            nc.vector.tensor_scalar_mul(that_sb[:], tu[:, 0:D], s2r[:])

            # ---- A^T and Bm^T (per half) + unpermuted outputs ----
            cpt = c_sb[:].rearrange("d (p t) -> d p t", t=NT)
            oa = outp.tile([D, LC], f32, tag="oa", name=f"oa{b}")
            oca = outp.tile([D, LC], f32, tag="oca", name=f"oca{b}")
            ocb = outp.tile([D, LC], f32, tag="ocb", name=f"ocb{b}")
            for g in range(2):
                gsl = slice(g * 512, (g + 1) * 512)
                tsl = slice(4 * g, 4 * g + 4)
                ath = mm_ps.tile([D, 512], f32, tag="mmh", name=f"at{b}_{g}")
                nc.tensor.matmul(ath[:], qt_sb[:], s1t[:, gsl],
                                 start=True, stop=True)
                bmh = mm_ps.tile([D, 512], f32, tag="mmh", name=f"bm{b}_{g}")
                nc.tensor.matmul(bmh[:], that_sb[:], s1t[:, gsl],
                                 start=True, stop=True)
                athp = ath[:].rearrange("d (t c) -> d c t", c=D)[:, 0:TL, :]
                bmhp = bmh[:].rearrange("d (t c) -> d c t", c=D)[:, 0:TL, :]
                oav = oa[:].rearrange("d (p t) -> d p t", t=NT)[:, :, tsl]
                ocav = oca[:].rearrange("d (p t) -> d p t", t=NT)[:, :, tsl]
                ocbv = ocb[:].rearrange("d (p t) -> d p t", t=NT)[:, :, tsl]
                cpg = cpt[:, :, tsl]
                nc.scalar.activation(oav, athp, AF.Copy)
                nc.vector.tensor_tensor(out=ocav, in0=cpg, in1=athp,
                                        op=ALU.mult)
                nc.vector.tensor_tensor(out=ocbv, in0=cpg, in1=bmhp,
                                        op=ALU.mult)
            nc.sync.dma_start(O_d[b, D:2 * D], oa[:])
            nc.sync.dma_start(O_d[b, 2 * D:3 * D], oca[:])
            nc.sync.dma_start(O_d[b, 3 * D:4 * D], ocb[:])

    nc.compile()
    return nc


def _get_nc(**kw):
    key = tuple(sorted(kw.items()))
    if key not in _cache:
        _cache[key] = _build(**kw)
    return _cache[key]


def kernel(C, Q, W, **build_kw):
    from concourse import bass_utils

    C = np.ascontiguousarray(C, np.float32)
    Q = np.ascontiguousarray(Q, np.float32)
    Wr = np.ascontiguousarray(W, np.float32).reshape(NCORES, NB, LC, 3 * D)
    Cs = C.reshape(NCORES, NB, D, LC)
    Qs = Q.reshape(NCORES, NB, D, LQ)

    nc = _get_nc(**build_kw)
    in_maps = [{"C": Cs[i], "Q": Qs[i], "W": Wr[i]} for i in range(NCORES)]
    res = bass_utils.run_bass_kernel_spmd(nc, in_maps,
                                          core_ids=list(range(NCORES)))
    out = np.concatenate([res.results[i]["OUT"] for i in range(NCORES)], 0)
    return out.astype(np.float32)


# revision 6
# speedup vs baseline: 1.3014x; 1.3014x over previous
"""CQAttention (context-query attention) Bass kernel for 8 NeuronCores.

Full inputs:  C [64,128,1000] f32, Q [64,128,100] f32, W [64000,1,384] f32
Full output:  [64, 512, 1000] f32

Sharding: pure data-parallel on the batch dim - 8 batches per core.

Per-batch math (D=128, Lc=1000, Lq=100):
  Ct = C.T [Lc,D], Qt = Q.T [Lq,D], w1/w2/w3 = W row blocks [Lc,D]
  U  = w1 + w3*Ct ; v = rowsum(w2*Ct)
  S  = U @ Q + v  (the v term drops out of the row softmax S1)
  S1 = softmax_cols(S) ; S2 = softmax_rows(S)
  A  = S1 @ Qt ; Bm = S1 @ (S2^T @ Ct)
  out = concat([Ct, A, Ct*A, Ct*Bm], 1).T  -> [4D, Lc]

Layout notes:
 - Lc is tiled 8 x 125 with the INTERLEAVED mapping i = p*8 + t (p =
   partition, t = tile) so the W DMA reads 12KB contiguous per partition.
   All intermediate tensors with an Lc axis are kept in the permuted
   (t-major) order; the final output ops unpermute via strided APs.
 - Scores are built transposed (S0T [Lq, Lc]); all big matmuls run with
   bf16 operands (full PE rate at any free size; transposes cost 1
   cycle/row instead of 2 for f32).
 - U^T is built with ONE bf16 PE transpose per tile: uraw = w1 + w3*Ct
   is folded on gpsimd/vector first.
 - s1 normalization: per-tile column sums of E1T via tiny [125,1]
   matmuls (so the reciprocal runs 128-lane-wide on a [125,8] tile
   instead of lane-starved on [1,1024]), then the scale rides the
   already-transposed e1p tiles; S1^T comes back via bf16 PE transposes.
 - S2 path: the same transposed e1p tiles scaled by exp(v), contracted
   with bf16 Ct tiles (ones column appended for the s2 sums).
 - Emission is software-pipelined: head(b+1) (loads/transposes of the
   next batch) is emitted before tail(b), so the PE queue always has
   independent work to fill cross-engine latency gaps.
 - DMA: all loads+stores on the sync hw-DGE queue; W is loaded as a 2D
   [125 x 12KB] pattern; loads are prefetched two batches ahead and the
   three computed output blocks go out as ONE 3D store.
"""

import numpy as np

B, D, LC, LQ = 64, 128, 1000, 100
NCORES = 8
NB = B // NCORES   # batches per core
NT = 8             # LC tiles
TL = LC // NT      # 125

_cache = {}


def _build():
    import concourse.bass as bass
    import concourse.tile as tile
    from concourse import bacc, mybir, masks
    from contextlib import ExitStack

    f32 = mybir.dt.float32
    bf16 = mybir.dt.bfloat16
    AF = mybir.ActivationFunctionType
    ALU = mybir.AluOpType
    AX = mybir.AxisListType

    nc = bacc.Bacc("TRN2", target_bir_lowering=False, debug=False,
                   num_devices=NCORES)
    C_d = nc.dram_tensor("C", [NB, D, LC], f32, kind="ExternalInput").ap()
    Q_d = nc.dram_tensor("Q", [NB, D, LQ], f32, kind="ExternalInput").ap()
    W_d = nc.dram_tensor("W", [NB, LC, 3 * D], f32, kind="ExternalInput").ap()
    O_d = nc.dram_tensor("OUT", [NB, 4 * D, LC], f32, kind="ExternalOutput").ap()

    with tile.TileContext(nc) as tc, ExitStack() as ctx:
        const_pool = ctx.enter_context(tc.tile_pool(name="const", bufs=1))
        ident = const_pool.tile([128, 128], f32)
        masks.make_identity(nc, ident[:])
        identb = const_pool.tile([128, 128], bf16)
        nc.scalar.activation(identb[:], ident[:], AF.Copy)
        ones_f = const_pool.tile([128, 1], f32)
        nc.vector.memset(ones_f[:], 1.0)
        ones_cb = const_pool.tile([128, 1], bf16)
        nc.scalar.activation(ones_cb[:], ones_f[:], AF.Copy)
        zero_f = const_pool.tile([128, 1], f32)
        nc.vector.memset(zero_f[:], 0.0)

        sb = ctx.enter_context(tc.tile_pool(name="sb", bufs=3))
        small = ctx.enter_context(tc.tile_pool(name="small", bufs=3))
        outp = ctx.enter_context(tc.tile_pool(name="outp", bufs=2))
        hp_ps = ctx.enter_context(tc.tile_pool(name="hp_ps", bufs=3, space="PSUM"))
        mm_ps = ctx.enter_context(tc.tile_pool(name="mm_ps", bufs=3, space="PSUM"))
        sm_ps = ctx.enter_context(tc.tile_pool(name="sm_ps", bufs=2, space="PSUM"))

        def load(b):
            q_sb = sb.tile([D, LQ], f32, tag="q", name=f"q{b}")
            nc.sync.dma_start(q_sb[:], Q_d[b])
            c_sb = sb.tile([D, LC], f32, tag="c", name=f"c{b}")
            nc.sync.dma_start(c_sb[:], C_d[b])
            # w_sb[p, t*3D+c] = W[b, p*8+t, c]  (12KB contiguous/partition)
            w_sb = sb.tile([TL, NT * 3 * D], f32, tag="w", name=f"w{b}")
            nc.sync.dma_start(
                w_sb[:], W_d[b].rearrange("(p t) c -> p (t c)", t=NT))
            # passthrough output rows 0:D = Ct (no compute dependency)
            nc.sync.dma_start(O_d[b, 0:D], c_sb[:])
            return w_sb, c_sb, q_sb

        def head(b, w_sb, c_sb, q_sb):
            """Loads -> Qt, Ct tiles, uraw = w1+w3*Ct, exp(v), U^T."""
            qb = small.tile([D, LQ], bf16, tag="qb", name=f"qb{b}")
            nc.scalar.activation(qb[:], q_sb[:], AF.Copy)
            qtp = hp_ps.tile([LQ, D], bf16, tag="tp", name=f"qtp{b}")
            nc.tensor.transpose(qtp[:], qb[:], identb[:])
            qt_sb = small.tile([LQ, D], bf16, tag="qt", name=f"qt{b}")
            nc.scalar.activation(qt_sb[:], qtp[:], AF.Copy)

            c_tiles = c_sb[:].rearrange("d (p t) -> d t p", t=NT)  # [D,t,p]
            wv = w_sb[:].rearrange("p (t c) -> p t c", c=3 * D)
            w1 = wv[:, :, 0:D]
            w2 = wv[:, :, D:2 * D]
            w3 = wv[:, :, 2 * D:3 * D]

            ct_sb = sb.tile([TL, NT * (D + 1)], bf16, tag="ct", name=f"ct{b}")
            ctv = ct_sb[:].rearrange("p (t c) -> p t c", c=D + 1)
            nc.vector.memset(ctv[:, :, D:D + 1], 1.0)
            w3ct = sb.tile([TL, NT * D], bf16, tag="w3ct", name=f"w3ct{b}")
            w3ctv = w3ct[:].rearrange("p (t c) -> p t c", c=D)
            uraw = sb.tile([TL, NT * D], bf16, tag="uraw", name=f"uraw{b}")
            urawv = uraw[:].rearrange("p (t c) -> p t c", c=D)
            for g in range(2):
                ctp = hp_ps.tile([TL, 4 * D], f32, tag="tp", name=f"ctp{b}_{g}")
                for k in range(4):
                    t = 4 * g + k
                    nc.tensor.transpose(
                        ctp[:, k * D:(k + 1) * D], c_tiles[:, t, :], ident[:])
                ctpv = ctp[:].rearrange("p (k c) -> p k c", c=D)
                gs = slice(4 * g, 4 * g + 4)
                nc.scalar.activation(ctv[:, gs, 0:D], ctpv, AF.Copy)
                nc.vector.tensor_tensor(out=w3ctv[:, gs, :], in0=w3[:, gs, :],
                                        in1=ctpv, op=ALU.mult)
                # uraw = w1 + w3*Ct (gpsimd, SBUF-only operands)
                nc.gpsimd.tensor_tensor(out=urawv[:, gs, :], in0=w1[:, gs, :],
                                        in1=w3ctv[:, gs, :], op=ALU.add)

            # v = rowsum(w2 * Ct); exp(v)
            vtmp = sb.tile([TL, NT * D], f32, tag="vtmp", name=f"vtmp{b}")
            vtmpv = vtmp[:].rearrange("p (t c) -> p t c", c=D)
            nc.gpsimd.tensor_tensor(out=vtmpv, in0=w2,
                                    in1=ctv[:, :, 0:D], op=ALU.mult)
            v_all = small.tile([TL, NT], f32, tag="v", name=f"v{b}")
            nc.vector.tensor_reduce(v_all[:], vtmpv, axis=AX.X, op=ALU.add)
            expv = small.tile([TL, NT], f32, tag="expv", name=f"expv{b}")
            nc.scalar.activation(expv[:], v_all[:], AF.Exp)

            # U^T via single bf16 PE transpose per tile
            ut_sb = sb.tile([D, 8 * D], bf16, tag="ut", name=f"ut{b}")
            utv = ut_sb[:].rearrange("d (t c) -> d t c", c=D)
            nc.scalar.activation(
                utv[:, :, TL:D],
                zero_f[:, 0:1].to_broadcast((D, NT, D - TL)), AF.Copy)
            for g in range(2):
                utp = hp_ps.tile([D, 4 * D], bf16, tag="tp", name=f"utp{b}_{g}")
                for k in range(4):
                    t = 4 * g + k
                    nc.tensor.transpose(
                        utp[:, k * D:k * D + TL],
                        uraw[:, t * D:(t + 1) * D],
                        identb[0:TL, 0:TL])
                nc.scalar.activation(
                    utv[:, 4 * g:4 * g + 4, 0:TL],
                    utp[:].rearrange("d (k c) -> d k c", c=D)[:, :, 0:TL],
                    AF.Copy)
            return dict(qb=qb, qt=qt_sb, ct=ct_sb, ctv=ctv, ut=ut_sb,
                        expv=expv, c=c_sb)

        def tail(b, H):
            qb, qt_sb, ctv, ut_sb, expv, c_sb = (
                H["qb"], H["qt"], H["ctv"], H["ut"], H["expv"], H["c"])

            # S0T = Qt @ U^T ; E1T = exp(S0T)
            e1t_sb = sb.tile([LQ, 8 * D], bf16, tag="e1t", name=f"e1t{b}")
            for g in range(2):
                s0g = mm_ps.tile([LQ, 512], f32, tag="mmh", name=f"s0t{b}_{g}")
                nc.tensor.matmul(s0g[:], qb[:],
                                 ut_sb[:, g * 512:(g + 1) * 512],
                                 start=True, stop=True)
                nc.scalar.activation(e1t_sb[:, g * 512:(g + 1) * 512],
                                     s0g[:], AF.Exp)

            # s1 normalization: per-tile column sums -> wide reciprocal
            csum = sm_ps.tile([TL, NT], f32, tag="sm", name=f"csum{b}")
            for t in range(NT):
                nc.tensor.matmul(csum[:, t:t + 1],
                                 e1t_sb[:, t * D:t * D + TL],
                                 ones_cb[0:LQ, :], start=True, stop=True)
            rinv = small.tile([TL, NT], f32, tag="rinv", name=f"rinv{b}")
            nc.vector.reciprocal(rinv[:], csum[:])

            # transposed E1T tiles; scale by exp(v) (-> E2 for the S2 path)
            # and by rinv (-> S1 rows, transposed back for the A/Bm path)
            e2_all = sb.tile([TL, NT * LQ], bf16, tag="e2", name=f"e2{b}")
            e2v = e2_all[:].rearrange("p (t c) -> p t c", c=LQ)
            s1p_all = sb.tile([TL, NT * LQ], bf16, tag="s1p", name=f"s1p{b}")
            s1pv = s1p_all[:].rearrange("p (t c) -> p t c", c=LQ)
            s1t = sb.tile([LQ, 8 * D], bf16, tag="s1t", name=f"s1t{b}")
            s1tv = s1t[:].rearrange("q (t c) -> q t c", c=D)
            nc.vector.memset(s1tv[:, :, TL:D], 0.0)
            for g in range(2):
                e1p = sm_ps.tile([TL, 4 * LQ], bf16, tag="sm",
                                 name=f"e1p{b}_{g}")
                for k in range(4):
                    t = 4 * g + k
                    nc.tensor.transpose(
                        e1p[:, k * LQ:(k + 1) * LQ],
                        e1t_sb[:, t * D:t * D + TL],
                        identb[0:LQ, 0:LQ])
                e1pv = e1p[:].rearrange("p (k c) -> p k c", c=LQ)
                gs = slice(4 * g, 4 * g + 4)
                scl = expv[:, gs].unsqueeze(-1).to_broadcast((TL, 4, LQ))
                nc.vector.tensor_tensor(out=e2v[:, gs, :], in0=e1pv,
                                        in1=scl, op=ALU.mult)
                rcl = rinv[:, gs].unsqueeze(-1).to_broadcast((TL, 4, LQ))
                nc.vector.tensor_tensor(out=s1pv[:, gs, :], in0=e1pv,
                                        in1=rcl, op=ALU.mult)
                s1tp = sm_ps.tile([LQ, 4 * D], bf16, tag="sm",
                                  name=f"s1tp{b}_{g}")
                for k in range(4):
                    t = 4 * g + k
                    nc.tensor.transpose(
                        s1tp[:, k * D:k * D + TL],
                        s1p_all[:, t * LQ:(t + 1) * LQ],
                        identb[0:TL, 0:TL])
                nc.scalar.activation(
                    s1tv[:, gs, 0:TL],
                    s1tp[:].rearrange("q (k c) -> q k c", c=D)[:, :, 0:TL],
                    AF.Copy)

            # Tu = E2^T @ [Ct | 1]  (accumulate over tiles)
            tu = sm_ps.tile([LQ, D + 1], f32, tag="sm", name=f"tu{b}")
            for t in range(NT):
                nc.tensor.matmul(tu[:], e2v[:, t, :], ctv[:, t, :],
                                 start=(t == 0), stop=(t == NT - 1))
            s2r = small.tile([LQ, 1], f32, tag="s2r", name=f"s2r{b}")
            nc.vector.reciprocal(s2r[:], tu[:, D:D + 1])
            that_sb = small.tile([LQ, D], bf16, tag="that", name=f"that{b}")
            nc.vector.tensor_scalar_mul(that_sb[:], tu[:, 0:D], s2r[:])

            # A^T and Bm^T (per half) + unpermuted outputs (one 3D store)
            cpt = c_sb[:].rearrange("d (p t) -> d p t", t=NT)
            oab = outp.tile([D, 3 * LC], f32, tag="oab", name=f"oab{b}")
            for g in range(2):
                gsl = slice(g * 512, (g + 1) * 512)
                tsl = slice(4 * g, 4 * g + 4)
                ath = mm_ps.tile([D, 512], f32, tag="mmh", name=f"at{b}_{g}")
                nc.tensor.matmul(ath[:], qt_sb[:], s1t[:, gsl],
                                 start=True, stop=True)
                bmh = mm_ps.tile([D, 512], f32, tag="mmh", name=f"bm{b}_{g}")
                nc.tensor.matmul(bmh[:], that_sb[:], s1t[:, gsl],
                                 start=True, stop=True)
                athp = ath[:].rearrange("d (t c) -> d c t", c=D)[:, 0:TL, :]
                bmhp = bmh[:].rearrange("d (t c) -> d c t", c=D)[:, 0:TL, :]
                oav = oab[:, 0:LC].rearrange(
                    "d (p t) -> d p t", t=NT)[:, :, tsl]
                ocav = oab[:, LC:2 * LC].rearrange(
                    "d (p t) -> d p t", t=NT)[:, :, tsl]
                ocbv = oab[:, 2 * LC:3 * LC].rearrange(
                    "d (p t) -> d p t", t=NT)[:, :, tsl]
                cpg = cpt[:, :, tsl]
                nc.scalar.activation(oav, athp, AF.Copy)
                nc.vector.tensor_tensor(out=ocav, in0=cpg, in1=athp,
                                        op=ALU.mult)
                nc.vector.tensor_tensor(out=ocbv, in0=cpg, in1=bmhp,
                                        op=ALU.mult)
            nc.sync.dma_start(
                O_d[b, D:4 * D].rearrange("(k d) c -> d k c", k=3),
                oab[:].rearrange("d (k c) -> d k c", k=3))

        tiles = {0: load(0)}
        if NB > 1:
            tiles[1] = load(1)
        H = {0: head(0, *tiles.pop(0))}
        for b in range(NB):
            if b + 2 < NB:
                tiles[b + 2] = load(b + 2)
            if b + 1 < NB:
                H[b + 1] = head(b + 1, *tiles.pop(b + 1))
            tail(b, H.pop(b))

    nc.compile()
    return nc


def _get_nc(**kw):
    key = tuple(sorted(kw.items()))
    if key not in _cache:
        _cache[key] = _build(**kw)
    return _cache[key]


def kernel(C, Q, W, **build_kw):
    from concourse import bass_utils

    C = np.ascontiguousarray(C, np.float32)
    Q = np.ascontiguousarray(Q, np.float32)
    Wr = np.ascontiguousarray(W, np.float32).reshape(NCORES, NB, LC, 3 * D)
    Cs = C.reshape(NCORES, NB, D, LC)
    Qs = Q.reshape(NCORES, NB, D, LQ)

    nc = _get_nc(**build_kw)
    in_maps = [{"C": Cs[i], "Q": Qs[i], "W": Wr[i]} for i in range(NCORES)]
    res = bass_utils.run_bass_kernel_spmd(nc, in_maps,
                                          core_ids=list(range(NCORES)))
    out = np.concatenate([res.results[i]["OUT"] for i in range(NCORES)], 0)
    return out.astype(np.float32)


# revision 8
# speedup vs baseline: 1.3577x; 1.0433x over previous
"""CQAttention (context-query attention) Bass kernel for 8 NeuronCores.

Full inputs:  C [64,128,1000] f32, Q [64,128,100] f32, W [64000,1,384] f32
Full output:  [64, 512, 1000] f32

Sharding: pure data-parallel on the batch dim - 8 batches per core.

Per-batch math (D=128, Lc=1000, Lq=100):
  Ct = C.T [Lc,D], Qt = Q.T [Lq,D], w1/w2/w3 = W row blocks [Lc,D]
  U  = w1 + w3*Ct ; v = rowsum(w2*Ct)
  S  = U @ Q + v  (the v term drops out of the row softmax S1)
  S1 = softmax_cols(S) ; S2 = softmax_rows(S)
  A  = S1 @ Qt ; Bm = S1 @ (S2^T @ Ct)
  out = concat([Ct, A, Ct*A, Ct*Bm], 1).T  -> [4D, Lc]

Layout notes:
 - Lc is tiled 8 x 125 with the INTERLEAVED mapping i = p*8 + t (p =
   partition, t = tile) so the W DMA reads 12KB contiguous per partition.
   All intermediate tensors with an Lc axis are kept in the permuted
   (t-major) order; the final output ops unpermute via strided APs.
 - Scores are built transposed (S0T [Lq, Lc]); all big matmuls run with
   bf16 operands (full PE rate at any free size; transposes cost 1
   cycle/row instead of 2 for f32).
 - U^T is built with ONE bf16 PE transpose per tile: uraw = w1 + w3*Ct
   is folded on gpsimd/vector first.
 - s1 normalization: per-tile column sums of E1T via tiny [125,1]
   matmuls (so the reciprocal runs 128-lane-wide on a [125,8] tile
   instead of lane-starved on [1,1024]); the scale rides the
   already-transposed e1p tiles; S1^T comes back via bf16 PE transposes.
 - S2 path: the same transposed e1p tiles scaled by exp(v), contracted
   with bf16 Ct tiles (ones column appended for the s2 sums).
 - Emission is software-pipelined 3 batches deep with sub-stage
   interleaving [S3(b) | S2a(b+1) | S1a(b+2) | S2b(b+1) | S1b(b+2)]:
   since every engine executes its queue in order, each stage's
   cross-engine stall window is covered by queued independent work from
   a neighbouring batch, and the PE rarely drops out of its high
   p-state.
 - DMA: all loads+stores on the sync hw-DGE queue; W is loaded as a 2D
   [125 x 12KB] pattern; loads run three batches ahead of use and the
   three computed output blocks go out as ONE 3D store.
"""

import numpy as np

B, D, LC, LQ = 64, 128, 1000, 100
NCORES = 8
NB = B // NCORES   # batches per core
NT = 8             # LC tiles
TL = LC // NT      # 125

_cache = {}


def _build():
    import concourse.bass as bass
    import concourse.tile as tile
    from concourse import bacc, mybir, masks
    from contextlib import ExitStack

    f32 = mybir.dt.float32
    bf16 = mybir.dt.bfloat16
    AF = mybir.ActivationFunctionType
    ALU = mybir.AluOpType
    AX = mybir.AxisListType

    nc = bacc.Bacc("TRN2", target_bir_lowering=False, debug=False,
                   num_devices=NCORES)
    C_d = nc.dram_tensor("C", [NB, D, LC], f32, kind="ExternalInput").ap()
    Q_d = nc.dram_tensor("Q", [NB, D, LQ], f32, kind="ExternalInput").ap()
    W_d = nc.dram_tensor("W", [NB, LC, 3 * D], f32, kind="ExternalInput").ap()
    O_d = nc.dram_tensor("OUT", [NB, 4 * D, LC], f32, kind="ExternalOutput").ap()

    with tile.TileContext(nc) as tc, ExitStack() as ctx:
        const_pool = ctx.enter_context(tc.tile_pool(name="const", bufs=1))
        ident = const_pool.tile([128, 128], f32)
        masks.make_identity(nc, ident[:])
        identb = const_pool.tile([128, 128], bf16)
        nc.scalar.activation(identb[:], ident[:], AF.Copy)
        ones_f = const_pool.tile([128, 1], f32)
        nc.vector.memset(ones_f[:], 1.0)
        ones_cb = const_pool.tile([128, 1], bf16)
        nc.scalar.activation(ones_cb[:], ones_f[:], AF.Copy)
        zero_f = const_pool.tile([128, 1], f32)
        nc.vector.memset(zero_f[:], 0.0)

        sb = ctx.enter_context(tc.tile_pool(name="sb", bufs=2))
        small = ctx.enter_context(tc.tile_pool(name="small", bufs=2))
        outp = ctx.enter_context(tc.tile_pool(name="outp", bufs=2))
        hp_ps = ctx.enter_context(tc.tile_pool(name="hp_ps", bufs=3, space="PSUM"))
        mm_ps = ctx.enter_context(tc.tile_pool(name="mm_ps", bufs=3, space="PSUM"))
        sm_ps = ctx.enter_context(tc.tile_pool(name="sm_ps", bufs=2, space="PSUM"))

        L = {}   # per-batch live tiles

        def load(b):
            d = {}
            d["q"] = sb.tile([D, LQ], f32, tag="q", bufs=3, name=f"q{b}")
            nc.sync.dma_start(d["q"][:], Q_d[b])
            d["c"] = sb.tile([D, LC], f32, tag="c", bufs=5, name=f"c{b}")
            nc.sync.dma_start(d["c"][:], C_d[b])
            # w_sb[p, t*3D+c] = W[b, p*8+t, c]  (12KB contiguous/partition)
            d["w"] = sb.tile([TL, NT * 3 * D], f32, tag="w", bufs=3,
                             name=f"w{b}")
            nc.sync.dma_start(
                d["w"][:], W_d[b].rearrange("(p t) c -> p (t c)", t=NT))
            # passthrough output rows 0:D = Ct (no compute dependency)
            nc.sync.dma_start(O_d[b, 0:D], d["c"][:])
            L[b] = d

        def s1a(b):
            """Qt, Ct tiles, w3ct/uraw/v chain, exp(v)."""
            d = L[b]
            w_sb, c_sb, q_sb = d["w"], d["c"], d["q"]
            d["qb"] = small.tile([D, LQ], bf16, tag="qb", bufs=3,
                                 name=f"qb{b}")
            nc.scalar.activation(d["qb"][:], q_sb[:], AF.Copy)
            qtp = hp_ps.tile([LQ, D], bf16, tag="tp", name=f"qtp{b}")
            nc.tensor.transpose(qtp[:], d["qb"][:], identb[:])
            d["qt"] = small.tile([LQ, D], bf16, tag="qt", bufs=4,
                                 name=f"qt{b}")
            nc.scalar.activation(d["qt"][:], qtp[:], AF.Copy)

            c_tiles = c_sb[:].rearrange("d (p t) -> d t p", t=NT)  # [D,t,p]
            wv = w_sb[:].rearrange("p (t c) -> p t c", c=3 * D)
            w1 = wv[:, :, 0:D]
            w2 = wv[:, :, D:2 * D]
            w3 = wv[:, :, 2 * D:3 * D]

            ct_sb = sb.tile([TL, NT * (D + 1)], bf16, tag="ct", bufs=4,
                            name=f"ct{b}")
            d["ct"] = ct_sb
            ctv = ct_sb[:].rearrange("p (t c) -> p t c", c=D + 1)
            d["ctv"] = ctv
            nc.vector.memset(ctv[:, :, D:D + 1], 1.0)
            w3ct = sb.tile([TL, NT * D], bf16, tag="w3ct", bufs=2,
                           name=f"w3ct{b}")
            w3ctv = w3ct[:].rearrange("p (t c) -> p t c", c=D)
            uraw = sb.tile([TL, NT * D], bf16, tag="uraw", bufs=2,
                           name=f"uraw{b}")
            d["uraw"] = uraw
            urawv = uraw[:].rearrange("p (t c) -> p t c", c=D)
            for g in range(2):
                ctp = hp_ps.tile([TL, 4 * D], f32, tag="tp", name=f"ctp{b}_{g}")
                for k in range(4):
                    t = 4 * g + k
                    nc.tensor.transpose(
                        ctp[:, k * D:(k + 1) * D], c_tiles[:, t, :], ident[:])
                ctpv = ctp[:].rearrange("p (k c) -> p k c", c=D)
                gs = slice(4 * g, 4 * g + 4)
                nc.scalar.activation(ctv[:, gs, 0:D], ctpv, AF.Copy)
                nc.vector.tensor_tensor(out=w3ctv[:, gs, :], in0=w3[:, gs, :],
                                        in1=ctpv, op=ALU.mult)
                # uraw = w1 + w3*Ct (gpsimd, SBUF-only operands)
                nc.gpsimd.tensor_tensor(out=urawv[:, gs, :], in0=w1[:, gs, :],
                                        in1=w3ctv[:, gs, :], op=ALU.add)

            # v = rowsum(w2 * Ct); exp(v)
            vtmp = sb.tile([TL, NT * D], f32, tag="vtmp", bufs=2,
                           name=f"vtmp{b}")
            vtmpv = vtmp[:].rearrange("p (t c) -> p t c", c=D)
            nc.gpsimd.tensor_tensor(out=vtmpv, in0=w2,
                                    in1=ctv[:, :, 0:D], op=ALU.mult)
            v_all = small.tile([TL, NT], f32, tag="v", bufs=2, name=f"v{b}")
            nc.vector.tensor_reduce(v_all[:], vtmpv, axis=AX.X, op=ALU.add)
            d["expv"] = small.tile([TL, NT], f32, tag="expv", bufs=3,
                                   name=f"expv{b}")
            nc.scalar.activation(d["expv"][:], v_all[:], AF.Exp)

        def s1b(b):
            """U^T via single bf16 PE transpose per tile."""
            d = L[b]
            ut_sb = sb.tile([D, 8 * D], bf16, tag="ut", bufs=3, name=f"ut{b}")
            d["ut"] = ut_sb
            utv = ut_sb[:].rearrange("d (t c) -> d t c", c=D)
            nc.scalar.activation(
                utv[:, :, TL:D],
                zero_f[:, 0:1].to_broadcast((D, NT, D - TL)), AF.Copy)
            for g in range(2):
                utp = hp_ps.tile([D, 4 * D], bf16, tag="tp", name=f"utp{b}_{g}")
                for k in range(4):
                    t = 4 * g + k
                    nc.tensor.transpose(
                        utp[:, k * D:k * D + TL],
                        d["uraw"][:, t * D:(t + 1) * D],
                        identb[0:TL, 0:TL])
                nc.scalar.activation(
                    utv[:, 4 * g:4 * g + 4, 0:TL],
                    utp[:].rearrange("d (k c) -> d k c", c=D)[:, :, 0:TL],
                    AF.Copy)

        def s2a(b):
            """S0T = Qt @ U^T ; E1T = exp(S0T)."""
            d = L[b]
            e1t = sb.tile([LQ, 8 * D], bf16, tag="e1t", bufs=2,
                          name=f"e1t{b}")
            d["e1t"] = e1t
            for g in range(2):
                s0g = mm_ps.tile([LQ, 512], f32, tag="mmh", name=f"s0t{b}_{g}")
                nc.tensor.matmul(s0g[:], d["qb"][:],
                                 d["ut"][:, g * 512:(g + 1) * 512],
                                 start=True, stop=True)
                nc.scalar.activation(e1t[:, g * 512:(g + 1) * 512],
                                     s0g[:], AF.Exp)

        def s2b(b):
            """s1 normalization + transposed tiles (E2 and S1^T)."""
            d = L[b]
            e1t = d["e1t"]
            # per-tile column sums -> wide reciprocal
            csum = sm_ps.tile([TL, NT], f32, tag="sm", name=f"csum{b}")
            for t in range(NT):
                nc.tensor.matmul(csum[:, t:t + 1],
                                 e1t[:, t * D:t * D + TL],
                                 ones_cb[0:LQ, :], start=True, stop=True)
            rinv = small.tile([TL, NT], f32, tag="rinv", bufs=2,
                              name=f"rinv{b}")
            nc.vector.reciprocal(rinv[:], csum[:])

            e2_all = sb.tile([TL, NT * LQ], bf16, tag="e2", bufs=3,
                             name=f"e2{b}")
            d["e2v"] = e2_all[:].rearrange("p (t c) -> p t c", c=LQ)
            s1p_all = sb.tile([TL, NT * LQ], bf16, tag="s1p", bufs=2,
                              name=f"s1p{b}")
            s1pv = s1p_all[:].rearrange("p (t c) -> p t c", c=LQ)
            s1t = sb.tile([LQ, 8 * D], bf16, tag="s1t", bufs=3,
                          name=f"s1t{b}")
            d["s1t"] = s1t
            s1tv = s1t[:].rearrange("q (t c) -> q t c", c=D)
            nc.vector.memset(s1tv[:, :, TL:D], 0.0)
            for g in range(2):
                e1p = sm_ps.tile([TL, 4 * LQ], bf16, tag="sm",
                                 name=f"e1p{b}_{g}")
                for k in range(4):
                    t = 4 * g + k
                    nc.tensor.transpose(
                        e1p[:, k * LQ:(k + 1) * LQ],
                        e1t[:, t * D:t * D + TL],
                        identb[0:LQ, 0:LQ])
                e1pv = e1p[:].rearrange("p (k c) -> p k c", c=LQ)
                gs = slice(4 * g, 4 * g + 4)
                scl = d["expv"][:, gs].unsqueeze(-1).to_broadcast((TL, 4, LQ))
                nc.vector.tensor_tensor(out=d["e2v"][:, gs, :], in0=e1pv,
                                        in1=scl, op=ALU.mult)
                rcl = rinv[:, gs].unsqueeze(-1).to_broadcast((TL, 4, LQ))
                nc.vector.tensor_tensor(out=s1pv[:, gs, :], in0=e1pv,
                                        in1=rcl, op=ALU.mult)
                s1tp = sm_ps.tile([LQ, 4 * D], bf16, tag="sm",
                                  name=f"s1tp{b}_{g}")
                for k in range(4):
                    t = 4 * g + k
                    nc.tensor.transpose(
                        s1tp[:, k * D:k * D + TL],
                        s1p_all[:, t * LQ:(t + 1) * LQ],
                        identb[0:TL, 0:TL])
                nc.scalar.activation(
                    s1tv[:, gs, 0:TL],
                    s1tp[:].rearrange("q (k c) -> q k c", c=D)[:, :, 0:TL],
                    AF.Copy)

        def s3(b):
            """Tu, That, A^T/Bm^T, outputs + one 3D store."""
            d = L[b]
            ctv, c_sb = d["ctv"], d["c"]
            tu = sm_ps.tile([LQ, D + 1], f32, tag="sm", name=f"tu{b}")
            for t in range(NT):
                nc.tensor.matmul(tu[:], d["e2v"][:, t, :], ctv[:, t, :],
                                 start=(t == 0), stop=(t == NT - 1))
            s2r = small.tile([LQ, 1], f32, tag="s2r", bufs=2, name=f"s2r{b}")
            nc.vector.reciprocal(s2r[:], tu[:, D:D + 1])
            that_sb = small.tile([LQ, D], bf16, tag="that", bufs=2,
                                 name=f"that{b}")
            nc.vector.tensor_scalar_mul(that_sb[:], tu[:, 0:D], s2r[:])

            cpt = c_sb[:].rearrange("d (p t) -> d p t", t=NT)
            oab = outp.tile([D, 3 * LC], f32, tag="oab", bufs=2,
                            name=f"oab{b}")
            for g in range(2):
                gsl = slice(g * 512, (g + 1) * 512)
                tsl = slice(4 * g, 4 * g + 4)
                ath = mm_ps.tile([D, 512], f32, tag="mmh", name=f"at{b}_{g}")
                nc.tensor.matmul(ath[:], d["qt"][:], d["s1t"][:, gsl],
                                 start=True, stop=True)
                bmh = mm_ps.tile([D, 512], f32, tag="mmh", name=f"bm{b}_{g}")
                nc.tensor.matmul(bmh[:], that_sb[:], d["s1t"][:, gsl],
                                 start=True, stop=True)
                athp = ath[:].rearrange("d (t c) -> d c t", c=D)[:, 0:TL, :]
                bmhp = bmh[:].rearrange("d (t c) -> d c t", c=D)[:, 0:TL, :]
                oav = oab[:, 0:LC].rearrange(
                    "d (p t) -> d p t", t=NT)[:, :, tsl]
                ocav = oab[:, LC:2 * LC].rearrange(
                    "d (p t) -> d p t", t=NT)[:, :, tsl]
                ocbv = oab[:, 2 * LC:3 * LC].rearrange(
                    "d (p t) -> d p t", t=NT)[:, :, tsl]
                cpg = cpt[:, :, tsl]
                nc.scalar.activation(oav, athp, AF.Copy)
                nc.vector.tensor_tensor(out=ocav, in0=cpg, in1=athp,
                                        op=ALU.mult)
                nc.vector.tensor_tensor(out=ocbv, in0=cpg, in1=bmhp,
                                        op=ALU.mult)
            nc.sync.dma_start(
                O_d[b, D:4 * D].rearrange("(k d) c -> d k c", k=3),
                oab[:].rearrange("d (k c) -> d k c", k=3))

        # ---- prologue ----
        for x in range(min(3, NB)):
            load(x)
        s1a(0)
        s1b(0)
        if NB > 1:
            s1a(1)
        s2a(0)
        s2b(0)
        if NB > 1:
            s1b(1)
        # ---- steady state ----
        for b in range(NB):
            if b + 3 < NB:
                load(b + 3)
            s3(b)
            if b + 1 < NB:
                s2a(b + 1)
            if b + 2 < NB:
                s1a(b + 2)
            if b + 1 < NB:
                s2b(b + 1)
            if b + 2 < NB:
                s1b(b + 2)
            L.pop(b)

    nc.compile()
    return nc


def _get_nc(**kw):
    key = tuple(sorted(kw.items()))
    if key not in _cache:
        _cache[key] = _build(**kw)
    return _cache[key]


def kernel(C, Q, W, **build_kw):
    from concourse import bass_utils

    C = np.ascontiguousarray(C, np.float32)
    Q = np.ascontiguousarray(Q, np.float32)
    Wr = np.ascontiguousarray(W, np.float32).reshape(NCORES, NB, LC, 3 * D)
    Cs = C.reshape(NCORES, NB, D, LC)
    Qs = Q.reshape(NCORES, NB, D, LQ)

    nc = _get_nc(**build_kw)
    in_maps = [{"C": Cs[i], "Q": Qs[i], "W": Wr[i]} for i in range(NCORES)]
    res = bass_utils.run_bass_kernel_spmd(nc, in_maps,
                                          core_ids=list(range(NCORES)))
    out = np.concatenate([res.results[i]["OUT"] for i in range(NCORES)], 0)
    return out.astype(np.float32)


# revision 11
# speedup vs baseline: 1.3846x; 1.0198x over previous
"""CQAttention (context-query attention) Bass kernel for 8 NeuronCores.

Full inputs:  C [64,128,1000] f32, Q [64,128,100] f32, W [64000,1,384] f32
Full output:  [64, 512, 1000] f32

Sharding: pure data-parallel on the batch dim - 8 batches per core.

Per-batch math (D=128, Lc=1000, Lq=100):
  Ct = C.T [Lc,D], Qt = Q.T [Lq,D], w1/w2/w3 = W row blocks [Lc,D]
  U  = w1 + w3*Ct ; v = rowsum(w2*Ct)
  S  = U @ Q + v  (the v term drops out of the row softmax S1)
  S1 = softmax_cols(S) ; S2 = softmax_rows(S)
  A  = S1 @ Qt ; Bm = S1 @ (S2^T @ Ct)
  out = concat([Ct, A, Ct*A, Ct*Bm], 1).T  -> [4D, Lc]

Layout notes:
 - Lc is tiled 8 x 125 with the INTERLEAVED mapping i = p*8 + t (p =
   partition, t = tile) so the W DMA reads 12KB contiguous per partition.
   All intermediate tensors with an Lc axis are kept in the permuted
   (t-major) order; the final output ops unpermute via strided APs.
 - Scores are built transposed (S0T [Lq, Lc]); all big matmuls run with
   bf16 operands (full PE rate at any free size; transposes cost 1
   cycle/row instead of 2 for f32).
 - U^T is built with ONE bf16 PE transpose per tile: uraw = w1 + w3*Ct
   is folded on gpsimd/vector first.
 - s1 normalization: per-tile column sums of E1T via tiny [125,1]
   matmuls (so the reciprocal runs 128-lane-wide on a [125,8] tile
   instead of lane-starved on [1,1024]); the scale rides the
   already-transposed e1p tiles; S1^T comes back via bf16 PE transposes.
 - S2 path: the same transposed e1p tiles scaled by exp(v), contracted
   with bf16 Ct tiles (ones column appended for the s2 sums).
 - Emission is software-pipelined 3 batches deep with sub-stage
   interleaving [S3(b) | S2a(b+1) | S1a(b+2) | S2b(b+1) | S1b(b+2)]:
   since every engine executes its queue in order, each stage's
   cross-engine stall window is covered by queued independent work from
   a neighbouring batch, and the PE rarely drops out of its high
   p-state.
 - DMA: all loads+stores on the sync hw-DGE queue; W is loaded as a 2D
   [125 x 12KB] pattern; loads run three batches ahead of use and the
   three computed output blocks go out as ONE 3D store.
"""

import numpy as np

B, D, LC, LQ = 64, 128, 1000, 100
NCORES = 8
NB = B // NCORES   # batches per core
NT = 8             # LC tiles
TL = LC // NT      # 125

_cache = {}


def _build():
    import concourse.bass as bass
    import concourse.tile as tile
    from concourse import bacc, mybir, masks
    from contextlib import ExitStack

    f32 = mybir.dt.float32
    bf16 = mybir.dt.bfloat16
    AF = mybir.ActivationFunctionType
    ALU = mybir.AluOpType
    AX = mybir.AxisListType

    nc = bacc.Bacc("TRN2", target_bir_lowering=False, debug=False,
                   num_devices=NCORES)
    C_d = nc.dram_tensor("C", [NB, D, LC], f32, kind="ExternalInput").ap()
    Q_d = nc.dram_tensor("Q", [NB, D, LQ], f32, kind="ExternalInput").ap()
    W_d = nc.dram_tensor("W", [NB, LC, 3 * D], f32, kind="ExternalInput").ap()
    O_d = nc.dram_tensor("OUT", [NB, 4 * D, LC], f32, kind="ExternalOutput").ap()

    with tile.TileContext(nc) as tc, ExitStack() as ctx:
        const_pool = ctx.enter_context(tc.tile_pool(name="const", bufs=1))
        ident = const_pool.tile([128, 128], f32)
        masks.make_identity(nc, ident[:])
        identb = const_pool.tile([128, 128], bf16)
        nc.scalar.activation(identb[:], ident[:], AF.Copy)
        ones_f = const_pool.tile([128, 1], f32)
        nc.vector.memset(ones_f[:], 1.0)
        ones_cb = const_pool.tile([128, 1], bf16)
        nc.scalar.activation(ones_cb[:], ones_f[:], AF.Copy)
        zero_f = const_pool.tile([128, 1], f32)
        nc.vector.memset(zero_f[:], 0.0)

        sb = ctx.enter_context(tc.tile_pool(name="sb", bufs=2))
        small = ctx.enter_context(tc.tile_pool(name="small", bufs=2))
        outp = ctx.enter_context(tc.tile_pool(name="outp", bufs=2))
        hp_ps = ctx.enter_context(tc.tile_pool(name="hp_ps", bufs=3, space="PSUM"))
        mm_ps = ctx.enter_context(tc.tile_pool(name="mm_ps", bufs=3, space="PSUM"))
        sm_ps = ctx.enter_context(tc.tile_pool(name="sm_ps", bufs=2, space="PSUM"))

        L = {}   # per-batch live tiles

        def load(b):
            d = {}
            d["q"] = sb.tile([D, LQ], f32, tag="q", bufs=3, name=f"q{b}")
            nc.sync.dma_start(d["q"][:], Q_d[b])
            d["c"] = sb.tile([D, LC], f32, tag="c", bufs=5, name=f"c{b}")
            nc.sync.dma_start(d["c"][:], C_d[b])
            # w_sb[p, t*3D+c] = W[b, p*8+t, c]  (12KB contiguous/partition)
            d["w"] = sb.tile([TL, NT * 3 * D], f32, tag="w", bufs=3,
                             name=f"w{b}")
            nc.sync.dma_start(
                d["w"][:], W_d[b].rearrange("(p t) c -> p (t c)", t=NT))
            # passthrough output rows 0:D = Ct (no compute dependency)
            nc.sync.dma_start(O_d[b, 0:D], d["c"][:])
            L[b] = d

        def s1a(b):
            """Qt, Ct tiles, w3ct/uraw/v chain, exp(v)."""
            d = L[b]
            w_sb, c_sb, q_sb = d["w"], d["c"], d["q"]
            d["qb"] = small.tile([D, LQ], bf16, tag="qb", bufs=3,
                                 name=f"qb{b}")
            nc.scalar.activation(d["qb"][:], q_sb[:], AF.Copy)
            qtp = hp_ps.tile([LQ, D], bf16, tag="tp", name=f"qtp{b}")
            nc.tensor.transpose(qtp[:], d["qb"][:], identb[:])
            d["qt"] = small.tile([LQ, D], bf16, tag="qt", bufs=4,
                                 name=f"qt{b}")
            nc.scalar.activation(d["qt"][:], qtp[:], AF.Copy)

            c_tiles = c_sb[:].rearrange("d (p t) -> d t p", t=NT)  # [D,t,p]
            wv = w_sb[:].rearrange("p (t c) -> p t c", c=3 * D)
            w1 = wv[:, :, 0:D]
            w2 = wv[:, :, D:2 * D]
            w3 = wv[:, :, 2 * D:3 * D]

            ct_sb = sb.tile([TL, NT * (D + 1)], bf16, tag="ct", bufs=4,
                            name=f"ct{b}")
            d["ct"] = ct_sb
            ctv = ct_sb[:].rearrange("p (t c) -> p t c", c=D + 1)
            d["ctv"] = ctv
            nc.vector.memset(ctv[:, :, D:D + 1], 1.0)
            w3ct = sb.tile([TL, NT * D], bf16, tag="w3ct", bufs=2,
                           name=f"w3ct{b}")
            w3ctv = w3ct[:].rearrange("p (t c) -> p t c", c=D)
            uraw = sb.tile([TL, NT * D], bf16, tag="uraw", bufs=3,
                           name=f"uraw{b}")
            d["uraw"] = uraw
            urawv = uraw[:].rearrange("p (t c) -> p t c", c=D)
            for g in range(2):
                ctp = hp_ps.tile([TL, 4 * D], f32, tag="tp", name=f"ctp{b}_{g}")
                for k in range(4):
                    t = 4 * g + k
                    nc.tensor.transpose(
                        ctp[:, k * D:(k + 1) * D], c_tiles[:, t, :], ident[:])
                ctpv = ctp[:].rearrange("p (k c) -> p k c", c=D)
                gs = slice(4 * g, 4 * g + 4)
                nc.scalar.activation(ctv[:, gs, 0:D], ctpv, AF.Copy)
                nc.vector.tensor_tensor(out=w3ctv[:, gs, :], in0=w3[:, gs, :],
                                        in1=ctpv, op=ALU.mult)
                # uraw = w1 + w3*Ct (gpsimd, SBUF-only operands)
                nc.gpsimd.tensor_tensor(out=urawv[:, gs, :], in0=w1[:, gs, :],
                                        in1=w3ctv[:, gs, :], op=ALU.add)

            # v = rowsum(w2 * Ct); exp(v)
            vtmp = sb.tile([TL, NT * D], f32, tag="vtmp", bufs=2,
                           name=f"vtmp{b}")
            vtmpv = vtmp[:].rearrange("p (t c) -> p t c", c=D)
            nc.gpsimd.tensor_tensor(out=vtmpv, in0=w2,
                                    in1=ctv[:, :, 0:D], op=ALU.mult)
            v_all = small.tile([TL, NT], f32, tag="v", bufs=2, name=f"v{b}")
            nc.vector.tensor_reduce(v_all[:], vtmpv, axis=AX.X, op=ALU.add)
            d["expv"] = small.tile([TL, NT], f32, tag="expv", bufs=3,
                                   name=f"expv{b}")
            nc.scalar.activation(d["expv"][:], v_all[:], AF.Exp)

        def s1b(b):
            """U^T via single bf16 PE transpose per tile."""
            d = L[b]
            ut_sb = sb.tile([D, 8 * D], bf16, tag="ut", bufs=3, name=f"ut{b}")
            d["ut"] = ut_sb
            utv = ut_sb[:].rearrange("d (t c) -> d t c", c=D)
            nc.scalar.activation(
                utv[:, :, TL:D],
                zero_f[:, 0:1].to_broadcast((D, NT, D - TL)), AF.Copy)
            for g in range(2):
                utp = hp_ps.tile([D, 4 * D], bf16, tag="tp", name=f"utp{b}_{g}")
                for k in range(4):
                    t = 4 * g + k
                    nc.tensor.transpose(
                        utp[:, k * D:k * D + TL],
                        d["uraw"][:, t * D:(t + 1) * D],
                        identb[0:TL, 0:TL])
                nc.scalar.activation(
                    utv[:, 4 * g:4 * g + 4, 0:TL],
                    utp[:].rearrange("d (k c) -> d k c", c=D)[:, :, 0:TL],
                    AF.Copy)

        def s2a(b):
            """S0T = Qt @ U^T ; E1T = exp(S0T)."""
            d = L[b]
            e1t = sb.tile([LQ, 8 * D], bf16, tag="e1t", bufs=2,
                          name=f"e1t{b}")
            d["e1t"] = e1t
            for g in range(2):
                s0g = mm_ps.tile([LQ, 512], f32, tag="mmh", name=f"s0t{b}_{g}")
                nc.tensor.matmul(s0g[:], d["qb"][:],
                                 d["ut"][:, g * 512:(g + 1) * 512],
                                 start=True, stop=True)
                nc.scalar.activation(e1t[:, g * 512:(g + 1) * 512],
                                     s0g[:], AF.Exp)

        def s2b(b):
            """s1 normalization + transposed tiles (E2 and S1^T)."""
            d = L[b]
            e1t = d["e1t"]
            # per-tile column sums -> wide reciprocal
            csum = sm_ps.tile([TL, NT], f32, tag="sm", name=f"csum{b}")
            for t in range(NT):
                nc.tensor.matmul(csum[:, t:t + 1],
                                 e1t[:, t * D:t * D + TL],
                                 ones_cb[0:LQ, :], start=True, stop=True)
            rinv = small.tile([TL, NT], f32, tag="rinv", bufs=2,
                              name=f"rinv{b}")
            nc.vector.reciprocal(rinv[:], csum[:])

            e2_all = sb.tile([TL, NT * LQ], bf16, tag="e2", bufs=3,
                             name=f"e2{b}")
            d["e2v"] = e2_all[:].rearrange("p (t c) -> p t c", c=LQ)
            s1p_all = sb.tile([TL, NT * LQ], bf16, tag="s1p", bufs=2,
                              name=f"s1p{b}")
            s1pv = s1p_all[:].rearrange("p (t c) -> p t c", c=LQ)
            s1t = sb.tile([LQ, 8 * D], bf16, tag="s1t", bufs=3,
                          name=f"s1t{b}")
            d["s1t"] = s1t
            s1tv = s1t[:].rearrange("q (t c) -> q t c", c=D)
            nc.vector.memset(s1tv[:, :, TL:D], 0.0)
            e1ps = []
            for g in range(2):
                e1p = sm_ps.tile([TL, 4 * LQ], bf16, tag="sm",
                                 name=f"e1p{b}_{g}")
                e1ps.append(e1p)
                for k in range(4):
                    t = 4 * g + k
                    nc.tensor.transpose(
                        e1p[:, k * LQ:(k + 1) * LQ],
                        e1t[:, t * D:t * D + TL],
                        identb[0:LQ, 0:LQ])
            for g in range(2):
                e1pv = e1ps[g][:].rearrange("p (k c) -> p k c", c=LQ)
                gs = slice(4 * g, 4 * g + 4)
                scl = d["expv"][:, gs].unsqueeze(-1).to_broadcast((TL, 4, LQ))
                nc.vector.tensor_tensor(out=d["e2v"][:, gs, :], in0=e1pv,
                                        in1=scl, op=ALU.mult)
                rcl = rinv[:, gs].unsqueeze(-1).to_broadcast((TL, 4, LQ))
                nc.vector.tensor_tensor(out=s1pv[:, gs, :], in0=e1pv,
                                        in1=rcl, op=ALU.mult)
            for g in range(2):
                gs = slice(4 * g, 4 * g + 4)
                s1tp = sm_ps.tile([LQ, 4 * D], bf16, tag="sm",
                                  name=f"s1tp{b}_{g}")
                for k in range(4):
                    t = 4 * g + k
                    nc.tensor.transpose(
                        s1tp[:, k * D:k * D + TL],
                        s1p_all[:, t * LQ:(t + 1) * LQ],
                        identb[0:TL, 0:TL])
                nc.scalar.activation(
                    s1tv[:, gs, 0:TL],
                    s1tp[:].rearrange("q (k c) -> q k c", c=D)[:, :, 0:TL],
                    AF.Copy)

        def s3(b):
            """Tu, That, A^T/Bm^T, outputs + one 3D store."""
            d = L[b]
            ctv, c_sb = d["ctv"], d["c"]
            tu = sm_ps.tile([LQ, D + 1], f32, tag="sm", name=f"tu{b}")
            for t in range(NT):
                nc.tensor.matmul(tu[:], d["e2v"][:, t, :], ctv[:, t, :],
                                 start=(t == 0), stop=(t == NT - 1))
            s2r = small.tile([LQ, 1], f32, tag="s2r", bufs=2, name=f"s2r{b}")
            nc.vector.reciprocal(s2r[:], tu[:, D:D + 1])
            that_sb = small.tile([LQ, D], bf16, tag="that", bufs=2,
                                 name=f"that{b}")
            nc.vector.tensor_scalar_mul(that_sb[:], tu[:, 0:D], s2r[:])

            cpt = c_sb[:].rearrange("d (p t) -> d p t", t=NT)
            oab = outp.tile([D, 3 * LC], f32, tag="oab", bufs=2,
                            name=f"oab{b}")
            for g in range(2):
                gsl = slice(g * 512, (g + 1) * 512)
                tsl = slice(4 * g, 4 * g + 4)
                ath = mm_ps.tile([D, 512], f32, tag="mmh", name=f"at{b}_{g}")
                nc.tensor.matmul(ath[:], d["qt"][:], d["s1t"][:, gsl],
                                 start=True, stop=True)
                bmh = mm_ps.tile([D, 512], f32, tag="mmh", name=f"bm{b}_{g}")
                nc.tensor.matmul(bmh[:], that_sb[:], d["s1t"][:, gsl],
                                 start=True, stop=True)
                athp = ath[:].rearrange("d (t c) -> d c t", c=D)[:, 0:TL, :]
                bmhp = bmh[:].rearrange("d (t c) -> d c t", c=D)[:, 0:TL, :]
                oav = oab[:, 0:LC].rearrange(
                    "d (p t) -> d p t", t=NT)[:, :, tsl]
                ocav = oab[:, LC:2 * LC].rearrange(
                    "d (p t) -> d p t", t=NT)[:, :, tsl]
                ocbv = oab[:, 2 * LC:3 * LC].rearrange(
                    "d (p t) -> d p t", t=NT)[:, :, tsl]
                cpg = cpt[:, :, tsl]
                nc.scalar.activation(oav, athp, AF.Copy)
                nc.vector.tensor_tensor(out=ocav, in0=cpg, in1=athp,
                                        op=ALU.mult)
                nc.vector.tensor_tensor(out=ocbv, in0=cpg, in1=bmhp,
                                        op=ALU.mult)
            nc.sync.dma_start(
                O_d[b, D:4 * D].rearrange("(k d) c -> d k c", k=3),
                oab[:].rearrange("d (k c) -> d k c", k=3))

        # ---- prologue ----
        for x in range(min(3, NB)):
            load(x)
        s1a(0)
        s1b(0)
        s2a(0)
        if NB > 1:
            s1a(1)
        s2b(0)
        # ---- steady state ----
        # Per iteration: everything emitted has its producers either in a
        # PRIOR iteration or earlier in this one, so every engine's
        # in-order queue almost never stalls:
        #   s3(b)     <- s2b(b)   [prev iter]
        #   s1b(b+1)  <- s1a(b+1) [prev iter, uraw ready on gpsimd]
        #   s2a(b+1)  <- s1b(b+1) [just emitted]
        #   s1a(b+2)  <- load     [3 iters ago]
        #   s2b(b+1)  <- s2a(b+1) [exp latency hidden by s1a(b+2) PE work]
        for b in range(NB):
            if b + 3 < NB:
                load(b + 3)
            s3(b)
            if b + 1 < NB:
                s1b(b + 1)
                s2a(b + 1)
            if b + 2 < NB:
                s1a(b + 2)
            if b + 1 < NB:
                s2b(b + 1)
            L.pop(b)

    nc.compile()
    return nc


def _get_nc(**kw):
    key = tuple(sorted(kw.items()))
    if key not in _cache:
        _cache[key] = _build(**kw)
    return _cache[key]


def kernel(C, Q, W, **build_kw):
    from concourse import bass_utils

    C = np.ascontiguousarray(C, np.float32)
    Q = np.ascontiguousarray(Q, np.float32)
    Wr = np.ascontiguousarray(W, np.float32).reshape(NCORES, NB, LC, 3 * D)
    Cs = C.reshape(NCORES, NB, D, LC)
    Qs = Q.reshape(NCORES, NB, D, LQ)

    nc = _get_nc(**build_kw)
    in_maps = [{"C": Cs[i], "Q": Qs[i], "W": Wr[i]} for i in range(NCORES)]
    res = bass_utils.run_bass_kernel_spmd(nc, in_maps,
                                          core_ids=list(range(NCORES)))
    out = np.concatenate([res.results[i]["OUT"] for i in range(NCORES)], 0)
    return out.astype(np.float32)


# revision 16
# speedup vs baseline: 1.5873x; 1.1464x over previous
"""CQAttention (context-query attention) Bass kernel for 8 NeuronCores.

Full inputs:  C [64,128,1000] f32, Q [64,128,100] f32, W [64000,1,384] f32
Full output:  [64, 512, 1000] f32

Sharding: pure data-parallel on the batch dim - 8 batches per core.

Per-batch math (D=128, Lc=1000, Lq=100):
  Ct = C.T [Lc,D], Qt = Q.T [Lq,D], w1/w2/w3 = W row blocks [Lc,D]
  U  = w1 + w3*Ct ; v = rowsum(w2*Ct)
  S  = U @ Q + v  (the v term drops out of the row softmax S1)
  S1 = softmax_cols(S) ; S2 = softmax_rows(S)
  A  = S1 @ Qt ; Bm = S1 @ (S2^T @ Ct)
  out = concat([Ct, A, Ct*A, Ct*Bm], 1).T  -> [4D, Lc]

Layout notes:
 - Lc is tiled 8 x 125 with the INTERLEAVED mapping i = p*8 + t (p =
   partition, t = tile) so the W DMA reads 12KB contiguous per partition.
   All intermediate tensors with an Lc axis are kept in the permuted
   (t-major) order; the final output ops unpermute via strided APs.
 - Scores are built transposed (S0T [Lq, Lc]); all big matmuls run with
   bf16 operands (full PE rate at any free size; transposes cost 1
   cycle/row instead of 2 for f32).
 - U^T is built with ONE bf16 PE transpose per tile: uraw = w1 + w3*Ct
   is folded on gpsimd/vector first.
 - s1 normalization: per-tile column sums of E1T via tiny [125,1]
   matmuls (so the reciprocal runs 128-lane-wide on a [125,8] tile
   instead of lane-starved on [1,1024]); the scale rides the
   already-transposed e1p tiles; S1^T comes back via bf16 PE transposes.
 - S2 path: the same transposed e1p tiles scaled by exp(v), contracted
   with bf16 Ct tiles (ones column appended for the s2 sums).
 - Emission is software-pipelined 3 batches deep with sub-stage
   interleaving [S3(b) | S2a(b+1) | S1a(b+2) | S2b(b+1) | S1b(b+2)]:
   since every engine executes its queue in order, each stage's
   cross-engine stall window is covered by queued independent work from
   a neighbouring batch, and the PE rarely drops out of its high
   p-state.
 - DMA: all loads+stores on the sync hw-DGE queue; W is loaded as a 2D
   [125 x 12KB] pattern; loads run three batches ahead of use and the
   three computed output blocks go out as ONE 3D store.
"""

import numpy as np

B, D, LC, LQ = 64, 128, 1000, 100
NCORES = 8
NB = B // NCORES   # batches per core
NT = 8             # LC tiles
TL = LC // NT      # 125

_cache = {}


def _build():
    import concourse.bass as bass
    import concourse.tile as tile
    from concourse import bacc, mybir, masks
    from contextlib import ExitStack

    f32 = mybir.dt.float32
    bf16 = mybir.dt.bfloat16
    AF = mybir.ActivationFunctionType
    ALU = mybir.AluOpType
    AX = mybir.AxisListType

    nc = bacc.Bacc("TRN2", target_bir_lowering=False, debug=False,
                   num_devices=NCORES)
    C_d = nc.dram_tensor("C", [NB, D, LC], f32, kind="ExternalInput").ap()
    Q_d = nc.dram_tensor("Q", [NB, D, LQ], f32, kind="ExternalInput").ap()
    W_d = nc.dram_tensor("W", [NB, LC, 3 * D], f32, kind="ExternalInput").ap()
    O_d = nc.dram_tensor("OUT", [NB, 4 * D, LC], f32, kind="ExternalOutput").ap()

    with tile.TileContext(nc) as tc, ExitStack() as ctx:
        const_pool = ctx.enter_context(tc.tile_pool(name="const", bufs=1))
        ident = const_pool.tile([128, 128], f32)
        masks.make_identity(nc, ident[:])
        identb = const_pool.tile([128, 128], bf16)
        nc.scalar.activation(identb[:], ident[:], AF.Copy)
        ones_f = const_pool.tile([128, 1], f32)
        nc.vector.memset(ones_f[:], 1.0)
        ones_cb = const_pool.tile([128, 1], bf16)
        nc.scalar.activation(ones_cb[:], ones_f[:], AF.Copy)
        zero_f = const_pool.tile([128, 1], f32)
        nc.vector.memset(zero_f[:], 0.0)

        sb = ctx.enter_context(tc.tile_pool(name="sb", bufs=2))
        small = ctx.enter_context(tc.tile_pool(name="small", bufs=2))
        outp = ctx.enter_context(tc.tile_pool(name="outp", bufs=2))
        hp_ps = ctx.enter_context(tc.tile_pool(name="hp_ps", bufs=3, space="PSUM"))
        mm_ps = ctx.enter_context(tc.tile_pool(name="mm_ps", bufs=3, space="PSUM"))
        sm_ps = ctx.enter_context(tc.tile_pool(name="sm_ps", bufs=2, space="PSUM"))

        L = {}   # per-batch live tiles

        def load(b):
            d = {}
            d["q"] = sb.tile([D, LQ], f32, tag="q", bufs=3, name=f"q{b}")
            nc.sync.dma_start(d["q"][:], Q_d[b])
            d["c"] = sb.tile([D, LC], f32, tag="c", bufs=5, name=f"c{b}")
            nc.sync.dma_start(d["c"][:], C_d[b])
            # w_sb[p, t*3D+c] = W[b, p*8+t, c]  (12KB contiguous/partition)
            d["w"] = sb.tile([TL, NT * 3 * D], f32, tag="w", bufs=3,
                             name=f"w{b}")
            # W rides the gpsimd SWDGE ring so its 1.5MB/batch never
            # head-of-line-blocks the sync queue's C/Q loads
            nc.gpsimd.dma_start(
                d["w"][:], W_d[b].rearrange("(p t) c -> p (t c)", t=NT))
            # passthrough output rows 0:D = Ct (no compute dependency)
            nc.sync.dma_start(O_d[b, 0:D], d["c"][:])
            L[b] = d

        def s1a1(b):
            """Qt, Ct tiles, w3ct."""
            d = L[b]
            w_sb, c_sb, q_sb = d["w"], d["c"], d["q"]
            d["qb"] = small.tile([D, LQ], bf16, tag="qb", bufs=3,
                                 name=f"qb{b}")
            nc.scalar.activation(d["qb"][:], q_sb[:], AF.Copy)
            qtp = hp_ps.tile([LQ, D], bf16, tag="tp", name=f"qtp{b}")
            nc.tensor.transpose(qtp[:], d["qb"][:], identb[:])
            d["qt"] = small.tile([LQ, D], bf16, tag="qt", bufs=4,
                                 name=f"qt{b}")
            nc.scalar.activation(d["qt"][:], qtp[:], AF.Copy)

            c_tiles = c_sb[:].rearrange("d (p t) -> d t p", t=NT)  # [D,t,p]
            wv = w_sb[:].rearrange("p (t c) -> p t c", c=3 * D)
            w1 = wv[:, :, 0:D]
            w2 = wv[:, :, D:2 * D]
            w3 = wv[:, :, 2 * D:3 * D]

            ct_sb = sb.tile([TL, NT * (D + 1)], bf16, tag="ct", bufs=4,
                            name=f"ct{b}")
            d["ct"] = ct_sb
            ctv = ct_sb[:].rearrange("p (t c) -> p t c", c=D + 1)
            d["ctv"] = ctv
            nc.vector.memset(ctv[:, :, D:D + 1], 1.0)
            w3ct = sb.tile([TL, NT * D], bf16, tag="w3ct", bufs=2,
                           name=f"w3ct{b}")
            d["w3ctv"] = w3ct[:].rearrange("p (t c) -> p t c", c=D)
            d["wv"] = wv
            for g in range(2):
                ctp = hp_ps.tile([TL, 4 * D], f32, tag="tp", name=f"ctp{b}_{g}")
                for k in range(4):
                    t = 4 * g + k
                    nc.tensor.transpose(
                        ctp[:, k * D:(k + 1) * D], c_tiles[:, t, :], ident[:])
                ctpv = ctp[:].rearrange("p (k c) -> p k c", c=D)
                gs = slice(4 * g, 4 * g + 4)
                nc.scalar.activation(ctv[:, gs, 0:D], ctpv, AF.Copy)
                nc.vector.tensor_tensor(out=d["w3ctv"][:, gs, :],
                                        in0=w3[:, gs, :],
                                        in1=ctpv, op=ALU.mult)

        def s1a2(b):
            """uraw = w1 + w3*Ct (gpsimd); v = rowsum(w2*Ct); exp(v)."""
            d = L[b]
            wv = d["wv"]
            w1 = wv[:, :, 0:D]
            w2 = wv[:, :, D:2 * D]
            ctv = d["ctv"]
            uraw = sb.tile([TL, NT * D], bf16, tag="uraw", bufs=3,
                           name=f"uraw{b}")
            d["uraw"] = uraw
            urawv = uraw[:].rearrange("p (t c) -> p t c", c=D)
            for g in range(2):
                gs = slice(4 * g, 4 * g + 4)
                nc.gpsimd.tensor_tensor(out=urawv[:, gs, :], in0=w1[:, gs, :],
                                        in1=d["w3ctv"][:, gs, :], op=ALU.add)
            vtmp = sb.tile([TL, NT * D], f32, tag="vtmp", bufs=2,
                           name=f"vtmp{b}")
            vtmpv = vtmp[:].rearrange("p (t c) -> p t c", c=D)
            nc.gpsimd.tensor_tensor(out=vtmpv, in0=w2,
                                    in1=ctv[:, :, 0:D], op=ALU.mult)
            v_all = small.tile([TL, NT], f32, tag="v", bufs=2, name=f"v{b}")
            nc.vector.tensor_reduce(v_all[:], vtmpv, axis=AX.X, op=ALU.add)
            d["expv"] = small.tile([TL, NT], f32, tag="expv", bufs=3,
                                   name=f"expv{b}")
            nc.scalar.activation(d["expv"][:], v_all[:], AF.Exp)

        def s1b(b):
            """U^T via single bf16 PE transpose per tile."""
            d = L[b]
            ut_sb = sb.tile([D, 8 * D], bf16, tag="ut", bufs=3, name=f"ut{b}")
            d["ut"] = ut_sb
            utv = ut_sb[:].rearrange("d (t c) -> d t c", c=D)
            nc.scalar.activation(
                utv[:, :, TL:D],
                zero_f[:, 0:1].to_broadcast((D, NT, D - TL)), AF.Copy)
            for g in range(2):
                utp = hp_ps.tile([D, 4 * D], bf16, tag="tp", name=f"utp{b}_{g}")
                for k in range(4):
                    t = 4 * g + k
                    nc.tensor.transpose(
                        utp[:, k * D:k * D + TL],
                        d["uraw"][:, t * D:(t + 1) * D],
                        identb[0:TL, 0:TL])
                nc.scalar.activation(
                    utv[:, 4 * g:4 * g + 4, 0:TL],
                    utp[:].rearrange("d (k c) -> d k c", c=D)[:, :, 0:TL],
                    AF.Copy)

        def s2a(b):
            """S0T = Qt @ U^T ; E1T = exp(S0T)."""
            d = L[b]
            e1t = sb.tile([LQ, 8 * D], bf16, tag="e1t", bufs=2,
                          name=f"e1t{b}")
            d["e1t"] = e1t
            for g in range(2):
                s0g = mm_ps.tile([LQ, 512], f32, tag="mmh", name=f"s0t{b}_{g}")
                nc.tensor.matmul(s0g[:], d["qb"][:],
                                 d["ut"][:, g * 512:(g + 1) * 512],
                                 start=True, stop=True)
                nc.scalar.activation(e1t[:, g * 512:(g + 1) * 512],
                                     s0g[:], AF.Exp)

        def s2b(b):
            """s1 normalization + transposed tiles (E2 and S1^T)."""
            d = L[b]
            e1t = d["e1t"]
            # per-tile column sums -> wide reciprocal
            csum = sm_ps.tile([TL, NT], f32, tag="sm", name=f"csum{b}")
            for t in range(NT):
                nc.tensor.matmul(csum[:, t:t + 1],
                                 e1t[:, t * D:t * D + TL],
                                 ones_cb[0:LQ, :], start=True, stop=True)
            rinv = small.tile([TL, NT], f32, tag="rinv", bufs=2,
                              name=f"rinv{b}")
            nc.vector.reciprocal(rinv[:], csum[:])

            e2_all = sb.tile([TL, NT * LQ], bf16, tag="e2", bufs=3,
                             name=f"e2{b}")
            d["e2v"] = e2_all[:].rearrange("p (t c) -> p t c", c=LQ)
            s1p_all = sb.tile([TL, NT * LQ], bf16, tag="s1p", bufs=2,
                              name=f"s1p{b}")
            s1pv = s1p_all[:].rearrange("p (t c) -> p t c", c=LQ)
            s1t = sb.tile([LQ, 8 * D], bf16, tag="s1t", bufs=3,
                          name=f"s1t{b}")
            d["s1t"] = s1t
            s1tv = s1t[:].rearrange("q (t c) -> q t c", c=D)
            nc.vector.memset(s1tv[:, :, TL:D], 0.0)
            e1ps = []
            for g in range(2):
                e1p = sm_ps.tile([TL, 4 * LQ], bf16, tag="sm",
                                 name=f"e1p{b}_{g}")
                e1ps.append(e1p)
                for k in range(4):
                    t = 4 * g + k
                    nc.tensor.transpose(
                        e1p[:, k * LQ:(k + 1) * LQ],
                        e1t[:, t * D:t * D + TL],
                        identb[0:LQ, 0:LQ])
            for g in range(2):
                e1pv = e1ps[g][:].rearrange("p (k c) -> p k c", c=LQ)
                gs = slice(4 * g, 4 * g + 4)
                scl = d["expv"][:, gs].unsqueeze(-1).to_broadcast((TL, 4, LQ))
                nc.vector.tensor_tensor(out=d["e2v"][:, gs, :], in0=e1pv,
                                        in1=scl, op=ALU.mult)
                rcl = rinv[:, gs].unsqueeze(-1).to_broadcast((TL, 4, LQ))
                nc.vector.tensor_tensor(out=s1pv[:, gs, :], in0=e1pv,
                                        in1=rcl, op=ALU.mult)
            for g in range(2):
                gs = slice(4 * g, 4 * g + 4)
                s1tp = sm_ps.tile([LQ, 4 * D], bf16, tag="sm",
                                  name=f"s1tp{b}_{g}")
                for k in range(4):
                    t = 4 * g + k
                    nc.tensor.transpose(
                        s1tp[:, k * D:k * D + TL],
                        s1p_all[:, t * LQ:(t + 1) * LQ],
                        identb[0:TL, 0:TL])
                nc.scalar.activation(
                    s1tv[:, gs, 0:TL],
                    s1tp[:].rearrange("q (k c) -> q k c", c=D)[:, :, 0:TL],
                    AF.Copy)

        def s3(b):
            """Tu, That, A^T/Bm^T, outputs + one 3D store."""
            d = L[b]
            ctv, c_sb = d["ctv"], d["c"]
            tu = sm_ps.tile([LQ, D + 1], f32, tag="sm", name=f"tu{b}")
            for t in range(NT):
                nc.tensor.matmul(tu[:], d["e2v"][:, t, :], ctv[:, t, :],
                                 start=(t == 0), stop=(t == NT - 1))
            s2r = small.tile([LQ, 1], f32, tag="s2r", bufs=2, name=f"s2r{b}")
            nc.vector.reciprocal(s2r[:], tu[:, D:D + 1])
            that_sb = small.tile([LQ, D], bf16, tag="that", bufs=2,
                                 name=f"that{b}")
            nc.vector.tensor_scalar_mul(that_sb[:], tu[:, 0:D], s2r[:])

            cpt = c_sb[:].rearrange("d (p t) -> d p t", t=NT)
            oab = outp.tile([D, 3 * LC], f32, tag="oab", bufs=2,
                            name=f"oab{b}")
            for g in range(2):
                gsl = slice(g * 512, (g + 1) * 512)
                tsl = slice(4 * g, 4 * g + 4)
                ath = mm_ps.tile([D, 512], f32, tag="mmh", name=f"at{b}_{g}")
                nc.tensor.matmul(ath[:], d["qt"][:], d["s1t"][:, gsl],
                                 start=True, stop=True)
                bmh = mm_ps.tile([D, 512], f32, tag="mmh", name=f"bm{b}_{g}")
                nc.tensor.matmul(bmh[:], that_sb[:], d["s1t"][:, gsl],
                                 start=True, stop=True)
                athp = ath[:].rearrange("d (t c) -> d c t", c=D)[:, 0:TL, :]
                bmhp = bmh[:].rearrange("d (t c) -> d c t", c=D)[:, 0:TL, :]
                oav = oab[:, 0:LC].rearrange(
                    "d (p t) -> d p t", t=NT)[:, :, tsl]
                ocav = oab[:, LC:2 * LC].rearrange(
                    "d (p t) -> d p t", t=NT)[:, :, tsl]
                ocbv = oab[:, 2 * LC:3 * LC].rearrange(
                    "d (p t) -> d p t", t=NT)[:, :, tsl]
                cpg = cpt[:, :, tsl]
                nc.vector.tensor_tensor(out=ocav, in0=cpg, in1=athp,
                                        op=ALU.mult)
                nc.vector.tensor_tensor(out=ocbv, in0=cpg, in1=bmhp,
                                        op=ALU.mult)
                nc.scalar.activation(oav, athp, AF.Copy)
            # store rides the scalar hw-DGE ring (sync carries C/Q loads)
            nc.scalar.dma_start(
                O_d[b, D:4 * D].rearrange("(k d) c -> d k c", k=3),
                oab[:].rearrange("d (k c) -> d k c", k=3))

        # ---- prologue ----
        for x in range(min(3, NB)):
            load(x)
        s1a1(0)
        s1a2(0)
        s1b(0)
        s2a(0)
        if NB > 1:
            s1a1(1)
            s1a2(1)
        s2b(0)
        # ---- steady state ----
        # Per iteration: everything emitted has its producers either in a
        # PRIOR iteration or earlier in this one, so every engine's
        # in-order queue almost never stalls:
        #   s3(b)      <- s2b(b)   [prev iter]
        #   s1b(b+1)   <- s1a2(b+1) [prev iter, uraw ready on gpsimd]
        #   s2a(b+1)   <- s1b(b+1)  [just emitted]
        #   s1a1(b+2)  <- load      [3 iters ago]; PE ct work hides the
        #                exp(b+1) latency; DVE w3ct lands mid-queue
        #   s2b(b+1)   <- s2a(b+1)
        #   s1a2(b+2)  <- s1a1(b+2) [gpsimd has the whole next iter]
        for b in range(NB):
            if b + 3 < NB:
                load(b + 3)
            s3(b)
            if b + 1 < NB:
                s1b(b + 1)
                s2a(b + 1)
            if b + 2 < NB:
                s1a1(b + 2)
            if b + 1 < NB:
                s2b(b + 1)
            if b + 2 < NB:
                s1a2(b + 2)
            L.pop(b)

    nc.compile()
    return nc


def _get_nc(**kw):
    key = tuple(sorted(kw.items()))
    if key not in _cache:
        _cache[key] = _build(**kw)
    return _cache[key]


def kernel(C, Q, W, **build_kw):
    from concourse import bass_utils

    C = np.ascontiguousarray(C, np.float32)
    Q = np.ascontiguousarray(Q, np.float32)
    Wr = np.ascontiguousarray(W, np.float32).reshape(NCORES, NB, LC, 3 * D)
    Cs = C.reshape(NCORES, NB, D, LC)
    Qs = Q.reshape(NCORES, NB, D, LQ)

    nc = _get_nc(**build_kw)
    in_maps = [{"C": Cs[i], "Q": Qs[i], "W": Wr[i]} for i in range(NCORES)]
    res = bass_utils.run_bass_kernel_spmd(nc, in_maps,
                                          core_ids=list(range(NCORES)))
    out = np.concatenate([res.results[i]["OUT"] for i in range(NCORES)], 0)
    return out.astype(np.float32)
